# revision 56
# baseline (speedup 1.0000x reference)
"""GPSA transformer block (ConViT-style) for TRN2, data-parallel over 8 cores.

Design notes (see git-style history in the optimization transcript):
 - ACT table-set discipline: softmax exp, the LN rstd (exp(-0.5*ln(var)))
   and the fused exp row-sums (accum_out) all live in the ln/exp table
   set; gelu is the only other set, loaded once per chunk (all 24 fc1
   gelus emitted contiguously). 8 table loads per run.
 - No LN stat DMA round trips: stats broadcast across partitions with a
   K=1 PE matmul (ones_row.T @ stat) + ACT copy PSUM->SBUF.
 - Softmax renorm folded into the PE transpose: rhs = diag(1/rowsum)
   instead of identity; row sums come free from the exp's accum_out. The
   gating pos-term (sqb, pre-scaled by sig/(1-sig), stored transposed) is
   added post-transpose by the DVE op that moves PSUM->SBUF.
 - Attention head loop software-pipelined in 3 phases (QK/exp+sums |
   diag+transpose+add | SV+O-copy); front(g+1) (LN1+qk/v, independent of
   chunk g) and the fc2(g-1) chains interleave as PE gap fillers, with
   q/k filler tiles ordered in (q,k) pairs so head h unblocks after
   2(h//2+1) projection blocks. PSUM rings: psS 2 / psT 2 / psO 1 /
   shared-mm 3 (the mm ring depth is what keeps filler chains flowing).
 - Dummy LDWEIGHTS at phase stall points reduce PE queue idling (they do
   not drive the HAM array-activity counter, so clock-gate throttling in
   the attention stretches remains the main residual cost).
 - q/k fp8, v/Sts bf16 (attention matmuls are moving-dim bound); all
   projections + fc1 fp8 DoubleRow (fc1 with per-row weight scales,
   dequanted via the gelu's per-partition scale AP); fc2 kept bf16 for
   accuracy. O fp8 (x16) with per-head gating (1-sig) folded into wproj;
   v_bias folded into the streamed x (combined-attention rows sum to 1).
 - All weight/activation loads are single coalesced DMAs (the engine DMA
   queues are latency-bound on instruction count, not bandwidth); x and
   its host-prepared bf16 copy live in DRAM as [128, KC, T] so one DMA
   covers a chunk. Queues: sync = xb/wqk/wproj, scalar = consts/sqbT/
   wfc1/wfc2/lnx, gpsimd = wqk tail/wv/y-stores.
"""
import sys

sys.path.insert(0, "/opt/trn_rl_repo")

import numpy as np
import ml_dtypes

import concourse.bass as bass
import concourse.mybir as mybir
from concourse import tile
from concourse.masks import make_identity

F32 = mybir.dt.float32
BF16 = mybir.dt.bfloat16
FP8 = mybir.dt.float8e4
AF = mybir.ActivationFunctionType
ALU = mybir.AluOpType

B, N, C, H = 64, 196, 768, 16
D = 48
DP = 64             # padded head dim
CP = H * DP         # 1024 padded attention channels
FF = 4 * C          # 3072
NCORES = 8
BLOC = B // NCORES  # 8 batches per core
T = BLOC * N        # 1568 tokens per core
TCH = 392           # token chunk = 2 batches
PADT = 400          # xn1 pair-tile free dim (16-aligned for DR ldweights)
NCHUNK = T // TCH   # 4
KC = C // 128       # 6
KP = KC // 2        # 3 fp8 k-pairs over C
KCP = CP // 128     # 8
KPP = KCP // 2      # 4 fp8 k-pairs over CP
KFF = FF // 128     # 24
SCALE = float(D) ** -0.5
EPS = 1e-5
SF = 64.0           # fp8 weight scale
SFO = 16.0          # fp8 O-activation scale (ungated rows sum to 1/(1-sig))
DR = mybir.MatmulPerfMode.DoubleRow

MAXW = 1  # walrus in this container rejects multi-wait instructions


class PatchedTileContext(tile.TileContext):
    """walrus in this container rejects >MAXW sync waits on one instruction
    ("Too many sync wait commands"). Split excess waits onto nofuse NoOps
    emitted just before the instruction on the same engine, and emit the
    exit-drain waits one per instruction."""

    def _add_instruction(self, inst):
        si = getattr(inst, "sync_info", None)
        waits = list(si.on_wait) if (si is not None and si.on_wait) else []
        if len(waits) > MAXW:
            extra = waits[:-MAXW]
            keep = waits[-MAXW:]
            for i in range(0, len(extra), MAXW):
                nop = mybir.InstNoOp(
                    name=f"{inst.name}_xw{i}",
                    sync_info=mybir.SyncInfo(
                        on_wait=extra[i : i + MAXW], on_update=[]),
                    bass_nofuse=True,
                    engine=inst.engine,
                )
                super()._add_instruction(nop)
            inst.sync_info = mybir.SyncInfo(
                on_wait=keep, on_update=list(si.on_update or []))
        super()._add_instruction(inst)

    def _drain_and_barrier(self, tick_clock, wait_clock):
        nc = self.nc
        clock = list(tick_clock.global_clock)
        for proc, sem in sorted(self.sems.allocated().items()):
            tick = clock[proc] if proc < len(clock) else 0
            if tick <= 0:
                continue
            mult = 16 if sem.name.startswith("DMA") else 1
            nc.sync.wait_ge(sem, tick * mult)
        nc.sync.drain()
        nc.all_engine_barrier()
        popped = nc._tile_sem_poison_stack.pop()
        assert popped is self._sem_poison
        nc.clear_and_free_semaphores(list(self.sems.allocated().values()))
        nc.all_engine_barrier()


def host_prep(inputs):
    """Preprocess full-problem numpy inputs into per-core DRAM tensor maps."""
    f32 = np.float32
    bf16 = ml_dtypes.bfloat16
    fp8 = ml_dtypes.float8_e4m3
    x = np.asarray(inputs["x"], f32)              # [B, N, C]
    n1w = np.asarray(inputs["norm1_w"], f32)
    n1b = np.asarray(inputs["norm1_b"], f32)
    n2w = np.asarray(inputs["norm2_w"], f32)
    n2b = np.asarray(inputs["norm2_b"], f32)
    qk_w = np.asarray(inputs["qk_w"], f32)        # [2C, C]
    v_w = np.asarray(inputs["v_w"], f32)          # [C, C]
    proj_w = np.asarray(inputs["proj_w"], f32)    # [C, C]
    proj_b = np.asarray(inputs["proj_b"], f32)
    fc1_w = np.asarray(inputs["fc1_w"], f32)      # [FF, C]
    fc1_b = np.asarray(inputs["fc1_b"], f32)
    fc2_w = np.asarray(inputs["fc2_w"], f32)      # [C, FF]
    fc2_b = np.asarray(inputs["fc2_b"], f32)

    sig = 1.0 / (1.0 + np.exp(-np.asarray(inputs["gating"], np.float64)))
    one_m_sig = (1.0 - sig).astype(np.float64)

    # padded q/k lhsT with LN1 weight folded, x64, fp8, k-pair layout
    wqk_pad = np.zeros((C, 2 * CP), f32)
    for h in range(H):
        wqk_pad[:, DP * h : DP * h + D] = (qk_w[D * h : D * h + D, :] * n1w[None, :]).T
        wqk_pad[:, CP + DP * h : CP + DP * h + D] = (
            qk_w[C + D * h : C + D * h + D, :] * n1w[None, :]).T
    wqk_p = (wqk_pad * SF).astype(fp8).reshape(KP, 2, 128, 2 * CP).transpose(2, 0, 1, 3).copy()
    qk_bias = qk_w @ n1b                          # [2C]
    qkb_pad = np.zeros((2 * CP,), f32)
    for h in range(H):
        qkb_pad[DP * h : DP * h + D] = qk_bias[D * h : D * h + D]
        qkb_pad[CP + DP * h : CP + DP * h + D] = qk_bias[C + D * h : C + D * h + D]
    qkb = qkb_pad.reshape(2 * KCP, 128).T.copy()  # [128, 16]

    wv_pad = np.zeros((C, CP), f32)
    for h in range(H):
        wv_pad[:, DP * h : DP * h + D] = (v_w[D * h : D * h + D, :] * n1w[None, :]).T
    wv_p = (wv_pad * SF).astype(fp8).reshape(KP, 2, 128, CP).transpose(2, 0, 1, 3).copy()
    v_bias = v_w @ n1b                            # [C]

    # proj lhsT with per-head (1-sig) gating folded into the rows
    wproj_pad = np.zeros((CP, C), f32)
    for h in range(H):
        wproj_pad[DP * h : DP * h + D, :] = (
            proj_w[:, D * h : D * h + D].T * float(one_m_sig[h]))
    wproj_p = (wproj_pad * SF).astype(fp8).reshape(KPP, 2, 128, C).transpose(2, 0, 1, 3).copy()

    # fc1 fp8 with per-output-row scales; dequant via ACT per-partition scale
    wfc1 = (fc1_w * n2w[None, :]).T               # [C, FF]
    absmax = np.abs(wfc1).max(axis=0)             # per FF-channel
    rs = 200.0 / np.maximum(absmax * SF, 1e-30)   # row scale on top of SF
    rs = np.minimum(rs, 8.0)                      # don't over-boost tiny rows
    wfc1_q = (wfc1 * (SF * rs)[None, :]).astype(fp8)
    wfc1_p = wfc1_q.reshape(KP, 2, 128, FF).transpose(2, 0, 1, 3).copy()
    fscale = (1.0 / (SF * rs)).astype(f32).reshape(KFF, 128).T.copy()  # [128, 24]
    fc1_beff = (fc1_b + fc1_w @ n2b).reshape(KFF, 128).T.copy()        # [128, 24]

    wfc2 = (fc2_w.T).astype(bf16)                 # [FF, C]
    wfc2_t = wfc2.reshape(KFF, 128, C).transpose(1, 0, 2).copy()

    # transposed, pre-scaled positional softmax: sqbT[h, m, n]
    s = int(N ** 0.5)
    ind = np.arange(s)[None, :] - np.arange(s)[:, None]
    indx = np.tile(ind, (s, s))
    indy = np.repeat(np.repeat(ind, s, axis=0), s, axis=1)
    rel = np.stack([indx, indy, indx ** 2 + indy ** 2], -1).astype(f32)
    logits = rel @ np.asarray(inputs["pos_w"], f32).T + np.asarray(inputs["pos_b"], f32)
    logits = logits.transpose(2, 0, 1).astype(np.float64)   # [H, N, N]
    e = np.exp(logits - logits.max(-1, keepdims=True))
    posP = e / e.sum(-1, keepdims=True)
    sqb = posP * (sig / np.maximum(one_m_sig, 1e-20))[:, None, None]
    sqbT = sqb.transpose(0, 2, 1)                                     # [H, m, n]
    sqbT1 = np.ascontiguousarray(sqbT[:, 0:128, :].transpose(1, 0, 2)).astype(bf16)
    sqbT2 = np.ascontiguousarray(sqbT[:, 128:196, :].transpose(1, 0, 2)).astype(bf16)

    common = {
        "wqk": wqk_p,
        "wv": wv_p,
        "wproj": wproj_p,
        "wfc1": wfc1_p,
        "wfc2": wfc2_t,
        "sqbT1": sqbT1,
        "sqbT2": sqbT2,
        "qkb": qkb,
        "fc1b": fc1_beff,
        "fscale": fscale,
    }
    # fold proj_b, fc2_b and the v-bias (through proj) into the streamed x:
    # combined attention rows sum to 1, so +v_bias on V becomes +proj_w@v_bias
    # on the output.
    xs_all = x + (proj_b + fc2_b + proj_w @ v_bias)[None, None, :]
    in_maps = []
    for c in range(NCORES):
        xs = xs_all[c * BLOC : (c + 1) * BLOC].reshape(T, C).T  # [C, T]
        xt = np.ascontiguousarray(
            xs.reshape(KC, 128, T).transpose(1, 0, 2))          # [128, KC, T]
        in_maps.append({"x": xt, "xb": xt.astype(bf16), **common})
    return in_maps


def build_bass():
    nc = bass.Bass()
    dram = {}
    for name, shape, dt in [
        ("x", [128, KC, T], F32),
        ("xb", [128, KC, T], BF16),
        ("wqk", [128, KP, 2, 2 * CP], FP8),
        ("wv", [128, KP, 2, CP], FP8),
        ("wproj", [128, KPP, 2, C], FP8),
        ("wfc1", [128, KP, 2, FF], FP8),
        ("wfc2", [128, KFF, C], BF16),
        ("sqbT1", [128, H, N], BF16),
        ("sqbT2", [68, H, N], BF16),
        ("qkb", [128, 2 * KCP], F32),
        ("fc1b", [128, KFF], F32),
        ("fscale", [128, KFF], F32),
    ]:
        dram[name] = nc.declare_dram_parameter(name, shape, dt, isOutput=False)
    y_d = nc.declare_dram_parameter("y", [C, T], F32, isOutput=True)

    with PatchedTileContext(nc) as tc:
        build_body(nc, tc, dram, y_d)
    return nc


def build_body(nc, tc, dram, y_d):
    consts = tc.alloc_tile_pool(name="consts", bufs=1)
    temps = tc.alloc_tile_pool(name="temps", bufs=2)

    ident = consts.tile([128, 128], BF16, tag="ident", name="ident")
    make_identity(nc, ident[:])
    ones_col = consts.tile([128, 1], BF16, tag="ones_col", name="ones_col")
    nc.vector.memset(ones_col[:], 1.0)
    ones_row = consts.tile([1, 128], BF16, tag="ones_row", name="ones_row")
    nc.vector.memset(ones_row[:], 1.0)
    eps_t = consts.tile([1, 1], F32, tag="eps_t", name="eps_t")
    nc.vector.memset(eps_t[:], EPS)
    small = {}
    for nm in ("qkb", "fc1b", "fscale"):
        t = consts.tile(list(dram[nm].shape), F32, tag=nm, name=nm)
        nc.scalar.dma_start(out=t[:], in_=dram[nm][:])
        small[nm] = t

    # ---- rolling activation pools (allocated before weights so chunk-0 x
    # prefetch can be emitted ahead of the big weight DMAs) ----
    p_lnx = tc.alloc_tile_pool(name="p_lnx", bufs=2)     # f32 x (LN1 + resid)
    p_xb = tc.alloc_tile_pool(name="p_xb", bufs=2)        # bf16 x copies
    p_xn1 = tc.alloc_tile_pool(name="p_xn1", bufs=4)      # fp8 pair tiles
    p_qk = tc.alloc_tile_pool(name="p_qk", bufs=16)       # bf16 q/k chunk tiles
    p_vt = tc.alloc_tile_pool(name="p_vt", bufs=3)        # bf16 vT per batch
    p_at = tc.alloc_tile_pool(name="p_at", bufs=2)        # attn temps
    p_dg = tc.alloc_tile_pool(name="p_dg", bufs=4)        # diag(rec) tiles
    p_o = tc.alloc_tile_pool(name="p_o", bufs=4)          # fp8 O pair tiles
    p_x1 = tc.alloc_tile_pool(name="p_x1", bufs=7)       # bf16 x1 chunk tiles
    p_xn2 = tc.alloc_tile_pool(name="p_xn2", bufs=6)      # fp8 xn2 pair tiles

    def load_xb(g):
        """One DMA for the chunk's bf16 x (LN1 input)."""
        xb = p_xb.tile([128, KC, TCH], BF16, tag="xb", name=f"xb{g}")
        nc.sync.dma_start(out=xb[:], in_=dram["xb"][:, :, bass.ts(g, TCH)])
        return [xb[:, ct, :] for ct in range(KC)]

    def load_lnx(g):
        """One DMA for the chunk's f32 x (kept for the proj residual)."""
        xt = p_lnx.tile([128, KC, TCH], F32, tag="lnx", name=f"lnx{g}")
        nc.scalar.dma_start(out=xt[:], in_=dram["x"][:, :, bass.ts(g, TCH)])
        return [xt[:, ct, :] for ct in range(KC)]

    # prefetch chunk 0+1 bf16 x before the weight DMAs hit the queues, so
    # the front(1) LN stats (emitted early in attention(0)) never stall the
    # in-order PE queue behind the weight transfers.
    xb_pre = {0: load_xb(0)}

    # ---- persistent weights, spread across engine DMA queues in need-order:
    # sync: wqk, wv, sqbT, wproj; scalar: wfc1, wfc2 ----
    pw = tc.alloc_tile_pool(name="pw", bufs=1)
    wqk_t = pw.tile([128, KP, 2, 2 * CP], FP8, tag="wqk", name="wqk")
    nc.sync.dma_start(out=wqk_t[:, 0:2], in_=dram["wqk"][:, 0:2])
    nc.gpsimd.dma_start(out=wqk_t[:, 2:3], in_=dram["wqk"][:, 2:3])
    wqk_sb = [wqk_t[:, p] for p in range(KP)]
    xb_pre[1] = load_xb(1)
    wv_t = pw.tile([128, KP, 2, CP], FP8, tag="wv", name="wv")
    nc.gpsimd.dma_start(out=wv_t[:], in_=dram["wv"][:])
    wv_sb = [wv_t[:, p] for p in range(KP)]
    sqb1 = pw.tile([128, H, N], BF16, tag="sqb1", name="sqb1")
    nc.scalar.dma_start(out=sqb1[:], in_=dram["sqbT1"][:])
    sqb2 = pw.tile([68, H, N], BF16, tag="sqb2", name="sqb2")
    nc.scalar.dma_start(out=sqb2[:], in_=dram["sqbT2"][:])
    sqb_sb = [(sqb1[:, h, :], sqb2[:, h, :]) for h in range(H)]
    wproj_t = pw.tile([128, KPP, 2, C], FP8, tag="wproj", name="wproj")
    nc.gpsimd.dma_start(out=wproj_t[:], in_=dram["wproj"][:])
    wproj_sb = [wproj_t[:, p] for p in range(KPP)]
    wfc1_t = pw.tile([128, KP, 2, FF], FP8, tag="wfc1", name="wfc1")
    nc.scalar.dma_start(out=wfc1_t[:], in_=dram["wfc1"][:])
    wfc1_sb = [wfc1_t[:, p] for p in range(KP)]
    wfc2_t = pw.tile([128, KFF, C], BF16, tag="wfc2", name="wfc2")
    nc.scalar.dma_start(out=wfc2_t[:], in_=dram["wfc2"][:])
    wfc2_sb = [wfc2_t[:, k] for k in range(KFF)]
    p_hdn = tc.alloc_tile_pool(name="p_hdn", bufs=24)     # bf16 hdn tiles

    ps_at = tc.alloc_tile_pool(name="ps_at", bufs=2, space="PSUM")
    ps_mm = tc.alloc_tile_pool(name="ps_mm", bufs=3, space="PSUM")

    def layernorm_stats(src_tiles):
        """PE stats + small-chain postprocess -> broadcast rstd_b/mr_b (bf16).
        src_tiles: 6 bf16 [128, TCH] tiles."""
        s1 = ps_mm.tile([1, TCH], F32, tag="mm", name="s1")
        s2 = ps_mm.tile([1, TCH], F32, tag="mm", name="s2")
        for ct in range(KC):
            xbt = src_tiles[ct]
            x2t = temps.tile([128, TCH], BF16, tag="x2t", name="x2t")
            eng = nc.gpsimd if ct % 2 == 0 else nc.vector
            eng.tensor_tensor(x2t[:], xbt[:], xbt[:], op=ALU.mult)
            nc.tensor.matmul(s1[:], ones_col[:], xbt[:],
                             start=(ct == 0), stop=(ct == KC - 1))
            nc.tensor.matmul(s2[:], ones_col[:], x2t[:],
                             start=(ct == 0), stop=(ct == KC - 1))
        mu = temps.tile([1, TCH], BF16, tag="mu", name="mu")
        nc.vector.tensor_scalar_mul(mu[:], s1[:], 1.0 / C)
        # scalar-stat chain packed along the free dim of one bf16 tile
        # (bf16 is safe: x is ~N(0,1) so ex2~1 >> mu^2, no cancellation)
        st4 = temps.tile([1, 4 * TCH], BF16, tag="st4", name="st4")
        ex2, mu2, var, lnv = (st4[:, i * TCH : (i + 1) * TCH] for i in range(4))
        nc.vector.tensor_scalar_mul(ex2, s2[:], 1.0 / C)
        nc.gpsimd.tensor_mul(mu2, mu[:], mu[:])
        nc.vector.tensor_sub(var, ex2, mu2)
        # rstd = exp(-0.5*ln(var+eps)): stays in the ln/exp ACT table set
        nc.scalar.activation(lnv, var, AF.Ln, bias=eps_t[:])
        rstd = temps.tile([1, TCH], BF16, tag="rstd", name="rstd")
        with nc.allow_low_precision(reason="bf16 rstd broadcast is accuracy-checked"):
            nc.scalar.activation(rstd[:], lnv, AF.Exp, scale=-0.5)
        mr = temps.tile([1, TCH], BF16, tag="mr", name="mr")
        nc.vector.tensor_mul(mr[:], mu[:], rstd[:])
        # broadcast across partitions via K=1 PE matmul (ones_row.T @ stat),
        # then ACT copy PSUM->SBUF bf16 (Copy lives in every ACT table set)
        bps = ps_mm.tile([128, TCH], F32, tag="mm", name="bps")
        nc.tensor.matmul(bps[:], ones_row[:], rstd[:], start=True, stop=True)
        mps = ps_mm.tile([128, TCH], F32, tag="mm", name="mps")
        nc.tensor.matmul(mps[:], ones_row[:], mr[:], start=True, stop=True)
        rstd_b = temps.tile([128, TCH], BF16, tag="rstd_b", name="rstd_b")
        nc.scalar.activation(rstd_b[:], bps[:], AF.Copy)
        mr_b = temps.tile([128, TCH], BF16, tag="mr_b", name="mr_b")
        nc.vector.tensor_scalar_mul(mr_b[:], mps[:], 1.0)
        return rstd_b, mr_b

    def front_closures(g):
        """front(g): LN1(g) + qk/v projections as a closure list, interleaved
        into attention(g-1). Only depends on the x(g) DMA."""
        work = []
        st = {"qkt": [None] * (2 * KCP), "vt": [[None, None], [None, None]]}

        def c_load():
            st["xbt"] = xb_pre.pop(g) if g in xb_pre else load_xb(g)
            st["lnx"] = load_lnx(g)

        def c_stats():
            st["bc"] = layernorm_stats(st["xbt"])

        def c_apply():
            rstd_b, mr_b = st["bc"]
            xn1g = [p_xn1.tile([128, 2, PADT], FP8, tag="xn1",
                               name=f"xn1_{g}_{p}") for p in range(KP)]
            for ct in range(KC):
                eng = nc.gpsimd if ct < 3 else nc.vector
                t1 = temps.tile([128, TCH], BF16, tag="t1", name="t1")
                eng.tensor_tensor(t1[:], st["xbt"][ct][:], rstd_b[:],
                                  op=ALU.mult)
                eng.tensor_tensor(xn1g[ct // 2][:, ct % 2, 0:TCH], t1[:],
                                  mr_b[:], op=ALU.subtract)
            st["xn1"] = xn1g

        work += [c_load, c_stats, c_apply]

        def c_qk(m):
            def run():
                ps = ps_mm.tile([128, TCH], F32, tag="mm", name="psqk")
                for p in range(KP):
                    nc.tensor.matmul(
                        ps[:], wqk_sb[p][:, :, bass.ts(m, 128)],
                        st["xn1"][p][:, :, 0:TCH],
                        start=(p == 0), stop=(p == KP - 1), perf_mode=DR)
                qt = p_qk.tile([128, TCH], FP8, tag="qk", name=f"qk{g}_{m}")
                if m < KCP:
                    nc.scalar.activation(qt[:], ps[:], AF.Identity,
                                         scale=1.0 / SF,
                                         bias=small["qkb"][:, m : m + 1])
                else:
                    nc.vector.tensor_scalar(qt[:], ps[:], 1.0 / SF,
                                            small["qkb"][:, m : m + 1],
                                            op0=ALU.mult, op1=ALU.add)
                st["qkt"][m] = qt
            return run

        def c_v(jb, half):
            def run():
                no, nn = ((0, 128), (128, 68))[half]
                tok0 = N * jb + no
                dst = p_vt.tile([nn, CP], BF16, tag=f"vt{nn}",
                                name=f"vt{g}_{jb}_{half}")
                for nch in range(2):
                    ps = ps_mm.tile([128, 512], F32, tag="mm", name="psv")
                    for p in range(KP):
                        nc.tensor.matmul(
                            ps[:nn], st["xn1"][p][:, :, tok0 : tok0 + nn],
                            wv_sb[p][:, :, bass.ts(nch, 512)],
                            start=(p == 0), stop=(p == KP - 1), perf_mode=DR)
                    nc.vector.tensor_scalar_mul(
                        dst[:, bass.ts(nch, 512)], ps[:nn], 1.0 / SF)
                st["vt"][jb][half] = dst
            return run

        for mq in range(KCP):
            work.append(c_qk(mq))
            work.append(c_qk(KCP + mq))
        for jb in range(2):
            for half in range(2):
                work.append(c_v(jb, half))
        return work, st

    def make_fc2(g, x1g, hdn, cs):
        """fc2(g) closure list: bf16 matmul chains + residual + store."""
        work = []

        def chain(m):
            def run():
                ps = ps_mm.tile([128, TCH], F32, tag="mm", name="psF2")
                for k in range(KFF):
                    nc.tensor.matmul(
                        ps[:], wfc2_sb[k][:, bass.ts(m, 128)], hdn[k][:],
                        start=(k == 0), stop=(k == KFF - 1))
                yt = temps.tile([128, TCH], F32, tag="yt", name="yt")
                nc.vector.tensor_tensor(yt[:], ps[:], x1g[m][:], op=ALU.add)
                nc.gpsimd.dma_start(out=y_d[bass.ts(m, 128), cs], in_=yt[:])
            return run

        for m in range(KC):
            work.append(chain(m))
        return work

    # ------------------------------------------------------------------
    fronts = {}
    w0, st0 = front_closures(0)
    fronts[0] = st0
    while w0:
        w0.pop(0)()
    fr_work, st1 = front_closures(1)
    fronts[1] = st1
    fc2_work = []

    for g in range(NCHUNK):
        cs = bass.ts(g, TCH)
        stg = fronts[g]
        qkt, vt = stg["qkt"], stg["vt"]

        # filler queue for the attention stretch: front(g+1) closures and
        # fc2(g-1) chains, merged so both spread across the head loop.
        filler = []
        fi, fj = 0, 0
        if fj < len(fc2_work):
            filler.append(fc2_work[fj]); fj += 1
        while fi < len(fr_work) or fj < len(fc2_work):
            for _ in range(4):
                if fi < len(fr_work):
                    filler.append(fr_work[fi]); fi += 1
            if fj < len(fc2_work):
                filler.append(fc2_work[fj]); fj += 1
        fr_work = []
        fc2_work = []
        pops_per_head = (len(filler) + H - 1) // H

        # ---------- attention: 3-phase software-pipelined head loop -------
        # phase1(h): QK + exp + rowsum + recip; phase2(h): diag + PE
        # transpose-with-renorm + pos add; phase3(h): SV + O store. Three
        # heads in flight keeps the PE fed through the softmax latency.
        Og = [p_o.tile([128, 2, TCH], FP8, tag="O", name=f"O{g}_{p}")
              for p in range(KPP)]
        hs = [dict() for _ in range(H)]

        def phase1(h):
            s = hs[h]
            qt = qkt[h // 2]
            kt = qkt[KCP + h // 2]
            ko = DP * (h % 2)
            psSa = ps_at.tile([128, 2, N], F32, tag="psS", name="psSa", bufs=2)
            psSb = ps_at.tile([68, 2, N], F32, tag="psS", name="psSb", bufs=2)
            for j in range(2):
                tb = N * j
                nc.tensor.matmul(
                    psSa[:, j, :], qt[ko : ko + DP, tb : tb + 128],
                    kt[ko : ko + DP, tb : tb + N], start=True, stop=True)
                nc.tensor.matmul(
                    psSb[:, j, :],
                    qt[ko : ko + DP, tb + 128 : tb + N],
                    kt[ko : ko + DP, tb : tb + N], start=True, stop=True)
            Ea = p_at.tile([128, 2, N], BF16, tag="Ea", name="Ea", bufs=3)
            Eb = p_at.tile([68, 2, N], BF16, tag="Eb", name="Eb", bufs=3)
            dens = p_at.tile([128, 4], F32, tag="dens", name="dens", bufs=4)
            for j in range(2):
                nc.scalar.activation(Ea[:, j, :], psSa[:, j, :], AF.Exp,
                                     scale=SCALE,
                                     accum_out=dens[:, j : j + 1])
                nc.scalar.activation(Eb[:, j, :], psSb[:, j, :], AF.Exp,
                                     scale=SCALE,
                                     accum_out=dens[0:68, 2 + j : 3 + j])
            rec = p_at.tile([128, 4], F32, tag="rec", name="rec", bufs=4)
            nc.vector.reciprocal(rec[:], dens[:])
            s["Ea"], s["Eb"], s["rec"] = Ea, Eb, rec

        def phase2(h):
            s = hs[h]
            Ea, Eb, rec = s["Ea"], s["Eb"], s["rec"]
            sq1, sq2 = sqb_sb[h]
            da, db = [], []
            for j in range(2):
                d1 = p_dg.tile([128, 128], BF16, tag="da", name="da")
                nc.vector.tensor_scalar(d1[:], ident[:], rec[:, j : j + 1],
                                        None, op0=ALU.mult)
                da.append(d1)
                d2 = p_dg.tile([68, 68], BF16, tag="db", name="db")
                nc.vector.tensor_scalar(d2[:], ident[0:68, 0:68],
                                        rec[0:68, 2 + j : 3 + j],
                                        None, op0=ALU.mult)
                db.append(d2)
            # dummy weight loads: keep the PE activity monitor (HAM) from
            # re-throttling the clock while the diag tiles are computed.
            # Harmless: every real matmul reloads its own weights.
            for _ in range(4):
                nc.tensor.ldweights(ident[:])
            psT = [ps_at.tile([128, TCH], F32, tag="psT", name=f"psT{j}")
                   for j in range(2)]
            for j in range(2):
                nc.tensor.matmul(psT[j][0:128, 0:128], Ea[:, j, 0:128],
                                 da[j][:], start=True, stop=True)
                nc.tensor.matmul(psT[j][0:128, 128:196], Eb[:, j, 0:128],
                                 db[j][:], start=True, stop=True)
                nc.tensor.matmul(psT[j][0:68, N : N + 128], Ea[:, j, 128:196],
                                 da[j][:], start=True, stop=True)
                nc.tensor.matmul(psT[j][0:68, N + 128 : 2 * N],
                                 Eb[:, j, 128:196], db[j][:],
                                 start=True, stop=True)
            Sts1 = p_at.tile([128, 2, N], BF16, tag="Sts1", name="Sts1",
                             bufs=3)
            Sts2 = p_at.tile([68, 2, N], BF16, tag="Sts2", name="Sts2",
                             bufs=3)
            for j in range(2):
                nc.vector.tensor_tensor(Sts1[:, j, :], psT[j][:, 0:N],
                                        sq1, op=ALU.add)
                nc.vector.tensor_tensor(Sts2[:, j, :],
                                        psT[j][0:68, N : 2 * N],
                                        sq2, op=ALU.add)
            s["Sts1"], s["Sts2"] = Sts1, Sts2

        def phase3(h):
            s = hs[h]
            for _ in range(4):
                nc.tensor.ldweights(ident[:])
            psO = ps_at.tile([DP, TCH], F32, tag="psO", name="psO", bufs=1)
            for j in range(2):
                js = bass.ds(N * j, N)
                v1, v2 = vt[j]
                nc.tensor.matmul(psO[:, js], v1[:, DP * h : DP * h + DP],
                                 s["Sts1"][:, j, :], start=True, stop=False)
                nc.tensor.matmul(psO[:, js], v2[:, DP * h : DP * h + DP],
                                 s["Sts2"][:, j, :], start=False, stop=True)
            nc.vector.tensor_scalar_mul(
                Og[h // 4][DP * (h % 2) : DP * (h % 2) + DP, (h // 2) % 2, :],
                psO[:], SFO)
            hs[h] = None

        pops_per_it = (len(filler) + H + 2) // (H + 3)
        for hh in range(H + 3):
            if hh < H:
                phase1(hh)
            for _ in range(pops_per_it):
                if filler:
                    filler.pop(0)()
            if 1 <= hh < H + 1:
                phase2(hh - 1)
            if hh >= 3:
                phase3(hh - 3)
        while filler:
            filler.pop(0)()

        # ---------- proj (DoubleRow fp8) + residual -> x1 bf16 ----------
        x1g = []
        for m in range(KC):
            ps = ps_mm.tile([128, TCH], F32, tag="mm", name="psP")
            for p in range(KPP):
                nc.tensor.matmul(
                    ps[:], wproj_sb[p][:, :, bass.ts(m, 128)], Og[p][:],
                    start=(p == 0), stop=(p == KPP - 1), perf_mode=DR)
            x1t = p_x1.tile([128, TCH], BF16, tag="x1", name=f"x1_{g}_{m}")
            nc.vector.scalar_tensor_tensor(
                x1t[:], ps[:], 1.0 / (SF * SFO), stg["lnx"][m][:],
                op0=ALU.mult, op1=ALU.add)
            x1g.append(x1t)

        # ---------- LN2(g) -> xn2 fp8 pairs (DVE apply) ----------
        rstd_b, mr_b = layernorm_stats(x1g)
        # dep-free dummy weight loads keep the PE clock warm through the
        # LN2 postprocess latency chain
        for _ in range(12):
            nc.tensor.ldweights(ident[:])
        xn2g = [p_xn2.tile([128, 2, TCH], FP8, tag="xn2",
                           name=f"xn2_{g}_{p}") for p in range(KP)]
        for ct in range(KC):
            t1 = temps.tile([128, TCH], BF16, tag="t2", name="t2")
            nc.vector.tensor_tensor(t1[:], x1g[ct][:], rstd_b[:], op=ALU.mult)
            nc.vector.tensor_tensor(xn2g[ct // 2][:, ct % 2, :], t1[:],
                                    mr_b[:], op=ALU.subtract)

        # ---------- fc1 (DoubleRow fp8) -> hdn bf16, gelus contiguous ------
        hdn = []
        for m in range(KFF):
            ps = ps_mm.tile([128, TCH], F32, tag="mm", name="psF1")
            for p in range(KP):
                nc.tensor.matmul(
                    ps[:], wfc1_sb[p][:, :, bass.ts(m, 128)], xn2g[p][:],
                    start=(p == 0), stop=(p == KP - 1), perf_mode=DR)
            ht = p_hdn.tile([128, TCH], BF16, tag="hdn", name=f"hdn{g}_{m}")
            nc.scalar.activation(ht[:], ps[:], AF.Gelu,
                                 scale=small["fscale"][:, m : m + 1],
                                 bias=small["fc1b"][:, m : m + 1])
            hdn.append(ht)
        fc2_work = make_fc2(g, x1g, hdn, cs)

        # stage front(g+2) for the next attention stretch
        if g + 2 < NCHUNK:
            fr_work, st_n = front_closures(g + 2)
            fronts[g + 2] = st_n
        else:
            fr_work = []

    while fc2_work:
        fc2_work.pop(0)()

    for pool in (ps_mm, ps_at, p_hdn, pw, p_xn2, p_x1, p_o, p_dg, p_at, p_vt,
                 p_qk, p_xn1, p_xb, p_lnx, temps, consts):
        pool.release()


def postprocess(results):
    """results: list of per-core out dicts with y [C, T] -> full [B, N, C]."""
    outs = []
    for c in range(NCORES):
        y = np.asarray(results[c]["y"])  # [C, T]
        outs.append(y.T.reshape(BLOC, N, C))
    return np.concatenate(outs, 0)


# ----------------------------------------------------------------------------
# Entry point: FULL inputs -> FULL output (8-core SPMD data-parallel).
# ----------------------------------------------------------------------------
_BUILD_CACHE = {}
LAST_RESULT = None


def kernel(**inputs) -> np.ndarray:
    global LAST_RESULT
    import os

    trace = os.environ.get("KERNEL_TRACE", "0") == "1"
    if trace:
        _install_ntff_shim()
    else:
        os.environ.setdefault("BASS_NEVER_TRACE", "1")
    from concourse.bass_utils import run_bass_kernel_spmd

    in_maps = host_prep(inputs)
    nc = _BUILD_CACHE.get("nc")
    if nc is None:
        nc = build_bass()
        _BUILD_CACHE["nc"] = nc
    kw = {}
    if trace:
        kw = dict(trace=True, tmpdir=os.environ.get("KERNEL_TRACE_DIR", None))
    res = run_bass_kernel_spmd(nc, in_maps, list(range(NCORES)), **kw)
    LAST_RESULT = res
    return postprocess(res.results)


def _install_ntff_shim():
    """Register the NTFF profile hook that this image's antenv lacks."""
    import types

    import antenv
    from concourse import bass_utils

    bass_utils.upload_artifacts = lambda tmpdir: f"local:{tmpdir}"
    if "antenv.axon_hooks" in sys.modules:
        return
    mod = types.ModuleType("antenv.axon_hooks")
    mod._hook = None
    mod.set_axon_ntff_profile_hook = lambda hook: setattr(mod, "_hook", hook)
    mod.get_axon_ntff_profile_hook = lambda: mod._hook
    sys.modules["antenv.axon_hooks"] = mod
    antenv.axon_hooks = mod
    from trn_agent_boot.trn_boot import _ntff_profile_via_ctypes

    hook = _ntff_profile_via_ctypes("/opt/axon/libaxon_pjrt.so")
    if hook is not None:
        mod.set_axon_ntff_profile_hook(hook)
